# revision 44
# baseline (speedup 1.0000x reference)
"""DeepSeek-MLA forward kernel for 8 Trainium2 NeuronCores (Bass/Tile).

Sharding: core c -> batch b = c // 4, head-group g = c % 4 (4 of 16 heads).
Each core computes its batch's down-projections, its 4 heads' attention,
and a partial output projection; the host sums the 4 partials per batch.

Design notes (restructured from the 344us baseline; ~210us measured):
- x is staged in HBM as the exact SBUF image [NSB, 128, NKC*SB] and pulled
  in 512KB linear transfers (the 8 cores share HBM at ~180GB/s/core, and
  sub-1MB strided transfers waste half of that).
- Phase 1 (down-proj) runs as one uninterrupted PE stream: the norm-stat
  ones-matmuls for block b are deferred until after block b+1's matmuls so
  the PE never waits on the ACT Square.
- KV-side RMS norm is deferred: c_kv stays unnormalized; 1/rms(c_kv) enters
  as the per-partition (per-key) scale vector of the softmax exp and as the
  per-partition scale of the V tiles (folded with the 1/8 softmax
  temperature).  Q-side norm is applied post-rope with a gpsimd-broadcast
  row multiply.
- All Ln activations run inline during phase 1 and the Exps are lumped
  after (Ln and Exp live in different act-table sets; interleaving costs
  ~1.3us table reloads).  rstdk8's Exp goes first (unblocks V scales),
  then block 0's rstdq slice, so the transition chain is short.  The
  softmax reciprocal runs on DVE (reciprocal_approx_fast; PSUM and nonzero
  base partitions are unsupported by custom DVE ops).
- Up-projection chunk order (q0, k0, v-mm, v-scale, q1, k1) matches the
  DVE FIFO so rope multiplies never queue behind rstd-gated scales.
- Attention runs per HEAD-PAIR PASS: scores for 2 heads land in one
  [128,1024] two-bank PSUM tile (one exp ACTIVATE per pair); a 3-deep
  score-tile ring lets the PE run a full wave ahead of the ACT exp stream.
- The next block's up-projection/rope chunks and the previous block's
  output-projection units are interleaved into the attention waves as PE
  filler, eliminating block-boundary stalls (causal path; dense/mask paths
  hoist all phase-1/up-projection work before attention).
- y partials are stored fp16 (halves DMA); host sums in fp32.
"""

import os
import sys

import numpy as np

for _p in ("/opt/trn_rl_repo", "/root/.axon_site/_ro/trn_rl_repo"):
    if os.path.isdir(_p) and _p not in sys.path:
        sys.path.insert(0, _p)

import concourse.bass as bass
import concourse.mybir as mybir
import concourse.tile as tile
from concourse import bacc

B, S, D, H, DN, DR, R = 2, 2048, 2048, 16, 32, 32, 128
HD = DN + DR  # 64
EPS = 1e-5
NCORES = 8
NH = 4          # heads per core
SB = 512        # s-block (psum bank width in f32)
NSB = S // SB   # 4
ST = 128        # s-tile
NST = S // ST   # 16
KC = 128        # contraction chunk
NKC = D // KC   # 16
VW = HD + 1     # v columns incl. eighth column (65)
F32 = mybir.dt.float32
BF16 = mybir.dt.bfloat16
F16 = mybir.dt.float16
F8E4 = mybir.dt.float8e4
LN_EIGHTH = float(np.log(0.125))
WO_SCALE = 64.0      # wo is pre-scaled x64 on host (fp8 range)
OUT_SCALE = 8.0      # outT stored x8 (fp8 range); y copy divides by 512


def _build_nc(causal: bool, use_mask: bool):
    nc = bacc.Bacc("TRN2", target_bir_lowering=False, debug=False,
                   num_devices=NCORES)

    # x stored as the exact SBUF image: [NSB, 128, NKC*SB] so each s-block
    # is ONE fully-linear 2MB DMA (16KB per partition row) -- ~340GB/s vs
    # ~170GB/s for 128KB chunked transfers.
    xT4 = nc.dram_tensor("xT4", [NSB, KC, NKC * SB], BF16,
                         kind="ExternalInput").ap()
    wkv = nc.dram_tensor("wkv", [KC, D], BF16, kind="ExternalInput").ap()
    wq = nc.dram_tensor("wq", [KC, D], BF16, kind="ExternalInput").ap()
    kb = nc.dram_tensor("kb", [R, 2 * KC], BF16, kind="ExternalInput").ap()
    ksh = nc.dram_tensor("ksh", [R, 2 * KC], BF16, kind="ExternalInput").ap()
    qb = nc.dram_tensor("qb", [R, 2 * KC], BF16, kind="ExternalInput").ap()
    qsh = nc.dram_tensor("qsh", [R, 2 * KC], BF16, kind="ExternalInput").ap()
    uv = nc.dram_tensor("uv", [R, NH * HD], BF16, kind="ExternalInput").ap()
    wo = nc.dram_tensor("wo", [KC, 2 * D], BF16, kind="ExternalInput").ap()
    # rope tables, s-block interleaved: per sb, cols [0:512]=cos, [512:1024]=sin
    cssin = nc.dram_tensor("cssin", [128, 2 * S], BF16, kind="ExternalInput").ap()
    maskT = None
    if use_mask:
        maskT = nc.dram_tensor("maskT", [S, S], F32, kind="ExternalInput").ap()
    y = nc.dram_tensor("y", [S, D], F16, kind="ExternalOutput").ap()
    dbg = {}
    if os.environ.get("MLA_DEBUG"):
        for nm, shp, dt in (("d_ckvT", [R, S], BF16), ("d_cqT", [R, S], BF16),
                            ("d_kT01", [128, S], BF16), ("d_kT23", [128, S], BF16),
                            ("d_qT01", [128, S], BF16), ("d_qT23", [128, S], BF16),
                            ("d_rstdq", [1, S], F32), ("d_rstdk8", [128, NST], F32),
                            ("d_vsb", [128, NST * NH * VW], BF16),
                            ):
            dbg[nm] = nc.dram_tensor(nm, shp, dt, kind="ExternalOutput").ap()

    AF = mybir.ActivationFunctionType
    ALU = mybir.AluOpType

    with tile.TileContext(nc) as tc:
        from contextlib import ExitStack
        with ExitStack() as ctx:
            stat = ctx.enter_context(tc.tile_pool(name="static", bufs=1))
            # persistent SBUF tensors
            xall = stat.tile([128, NKC * S], BF16, name="xall")
            ckvT = stat.tile([R, S], BF16, name="ckvT")
            cqT = stat.tile([R, S], BF16, name="cqT")
            kT01 = stat.tile([128, S], BF16, name="kT01")
            kT23 = stat.tile([128, S], BF16, name="kT23")
            qT01 = stat.tile([128, S], BF16, name="qT01")
            qT23 = stat.tile([128, S], BF16, name="qT23")
            v_sb = stat.tile([128, NST * NH * VW], BF16, name="v_sb")
            outT01 = stat.tile([128, S], BF16, name="outT01")
            outT23 = stat.tile([128, S], BF16, name="outT23")
            wkv_sb = stat.tile([KC, D], BF16, name="wkv_sb")
            wq_sb = stat.tile([KC, D], BF16, name="wq_sb")
            kb_sb = stat.tile([R, 2 * KC], BF16, name="kb_sb")
            ksh_sb = stat.tile([R, 2 * KC], BF16, name="ksh_sb")
            qb_sb = stat.tile([R, 2 * KC], BF16, name="qb_sb")
            qsh_sb = stat.tile([R, 2 * KC], BF16, name="qsh_sb")
            uv_sb = stat.tile([R, NH * HD], BF16, name="uv_sb")
            wo_sb = stat.tile([KC, 2 * D], BF16, name="wo_sb")
            cssin_sb = stat.tile([128, 2 * S], BF16, name="cssin_sb")
            tri_sb = stat.tile([128, 128], BF16, name="tri_sb")
            ones_col = stat.tile([128, 1], BF16, name="ones_col")
            rstdk8 = stat.tile([128, NST], F32, name="rstdk8")
            rstdq = stat.tile([1, S], F32, name="rstdq")
            rstdkv = stat.tile([1, S], F32, name="rstdkv") if use_mask else None
            msq_row = stat.tile([1, S], F32, name="msq_row")
            mskv_row = stat.tile([1, S], F32, name="mskv_row") if use_mask \
                else None
            msT_all = stat.tile([128, NST], F32, name="msT_all")
            eps_sb = stat.tile([128, 1], F32, name="eps_sb")
            ln8_sb = stat.tile([128, 1], F32, name="ln8_sb")

            # ---- static loads, ordered by first use: wkv/wq + x block 0
            # feed phase 1 immediately; wo (first used ~40us in) goes last ----
            nc.sync.dma_start(wkv_sb[:], wkv)
            nc.sync.dma_start(wq_sb[:], wq)

            def load_x(sb, parts=1):
                w = NKC * SB // parts
                for p in range(parts):
                    nc.sync.dma_start(
                        xall[:, sb * NKC * SB + p * w:
                             sb * NKC * SB + (p + 1) * w],
                        xT4[sb, :, p * w:(p + 1) * w])
            # x in 4 sub-transfers per block so phase 1 streams; the
            # up-projection weights slot between x0's head and tail parts
            # so attention j=0 can start as soon as block 0 is down.
            def load_x_part(sb, p):
                w = NKC * SB // 4
                nc.sync.dma_start(
                    xall[:, sb * NKC * SB + p * w:
                         sb * NKC * SB + (p + 1) * w],
                    xT4[sb, :, p * w:(p + 1) * w])
            load_x_part(0, 0)
            load_x_part(0, 1)
            nc.sync.dma_start(kb_sb[:], kb)
            nc.sync.dma_start(ksh_sb[:], ksh)
            nc.sync.dma_start(qb_sb[:], qb)
            nc.sync.dma_start(qsh_sb[:], qsh)
            nc.sync.dma_start(cssin_sb[:], cssin)
            nc.sync.dma_start(uv_sb[:], uv)
            load_x_part(0, 2)
            load_x_part(0, 3)
            for sb in range(1, NSB):
                for p in range(4):
                    load_x_part(sb, p)
            nc.sync.dma_start(wo_sb[:], wo)

            nc.gpsimd.memset(ones_col[:], 1.0)
            nc.gpsimd.memset(tri_sb[:], 1.0)
            nc.gpsimd.affine_select(
                out=tri_sb[:], in_=tri_sb[:], compare_op=ALU.is_ge,
                fill=0.0, base=0, channel_multiplier=-1, pattern=[[1, 128]])
            nc.gpsimd.memset(eps_sb[:], EPS)
            nc.gpsimd.memset(ln8_sb[:], LN_EIGHTH)
            # v pre-filled with 0.125: PV accumulates denom/8; the DVE
            # reciprocal yields 8/denom, un-doing the 1/8 temperature that is
            # folded into the V scale.  (Last: it's big and not needed until
            # the first PV wave.)
            nc.gpsimd.memset(v_sb[:], 0.125)

            v_blocks = v_sb.rearrange("p (t h w) -> p t h w", t=NST, h=NH)

            # pools.  PSUM budget (8 banks): pair 2x2 + ph1 1x2 (cps/small,
            # own tag so a long-lived cps interleaved into attention waves
            # does not collapse the score-tile ring) + oacc 2x1.
            pair = ctx.enter_context(
                tc.tile_pool(name="pair", bufs=2, space="PSUM"))
            oaccp = ctx.enter_context(
                tc.tile_pool(name="oaccp", bufs=2, space="PSUM"))
            sqp = ctx.enter_context(tc.tile_pool(name="sqp", bufs=1 if use_mask else 2))
            t12p = ctx.enter_context(tc.tile_pool(name="t12p", bufs=1 if use_mask else 2))
            etp = ctx.enter_context(tc.tile_pool(name="etp", bufs=2 if use_mask else 4))
            rcp = ctx.enter_context(tc.tile_pool(name="rcp", bufs=2))
            tinyp = ctx.enter_context(tc.tile_pool(name="tinyp", bufs=2))
            rbqp = ctx.enter_context(tc.tile_pool(name="rbqp", bufs=2))
            rbop = ctx.enter_context(tc.tile_pool(name="rbop", bufs=2))
            ysbp = ctx.enter_context(tc.tile_pool(name="ysbp", bufs=1 if use_mask else 3))
            tailp = None if use_mask else ctx.enter_context(tc.tile_pool(name="tailp", bufs=1))
            ocp = ctx.enter_context(tc.tile_pool(name="ocp", bufs=2 if use_mask else 4))
            # mask path: reclaim 4kb from etp

            mtp = ctx.enter_context(tc.tile_pool(name="mtp", bufs=1)) \
                if use_mask else None

            # ================= Fused P2 -> P3 -> P4 pipeline ==============
            def p2_chunks(sb):
                """Up-projection + rope for block sb as filler closures, so
                block sb's kT/qT/v are produced during block sb-1's waves."""
                sl = slice(sb * SB, (sb + 1) * SB)
                cs_sl = cssin_sb[:, sb * 2 * SB:(sb + 1) * 2 * SB]
                state = {}

                def bcast():
                    rbq = rbqp.tile([128, SB], F32, name="rbq", tag="rbq")
                    nc.gpsimd.partition_broadcast(rbq[:], rstdq[0:1, sl])
                    state["rbq"] = rbq
                    if use_mask:
                        rbkv = rbqp.tile([128, SB], F32, name="rbkv",
                                         tag="rbq")
                        nc.gpsimd.partition_broadcast(rbkv[:],
                                                      rstdkv[0:1, sl])
                        state["rbkv"] = rbkv

                def group(cT, wb, wsh, dsts, rbkey, p):
                    def emit():
                        pp = pair.tile([128, 2 * SB], F32, name="pp",
                                       tag="pair")
                        nc.tensor.matmul(pp[:, 0:SB],
                                         wb[:, p * KC:(p + 1) * KC], cT[:, sl])
                        nc.tensor.matmul(pp[:, SB:2 * SB],
                                         wsh[:, p * KC:(p + 1) * KC], cT[:, sl])
                        t12 = t12p.tile([128, 2 * SB], F32, name="t12",
                                        tag="t12")
                        nc.vector.tensor_mul(t12[:], pp[:], cs_sl)
                        nc.vector.tensor_add(dsts[p][:, sl], t12[:, 0:SB],
                                             t12[:, SB:2 * SB])
                        rb = state.get(rbkey)
                        if rb is not None:
                            nc.vector.tensor_mul(dsts[p][:, sl],
                                                 dsts[p][:, sl], rb[:])
                    return emit

                def vtiles_mm():
                    vps = pair.tile([128, 2 * SB], F32, name="vps", tag="pair")
                    for t in range(4):
                        g = sb * 4 + t
                        nc.tensor.matmul(vps[:, t * 256:(t + 1) * 256],
                                         ckvT[:, g * ST:(g + 1) * ST],
                                         uv_sb[:])
                    state["vps"] = vps

                def vtiles_scale():
                    vps = state["vps"]
                    for t in range(4):
                        g = sb * 4 + t
                        vsrc = vps[:, t * 256:(t + 1) * 256].rearrange(
                            "p (h d) -> p h d", h=NH)
                        nc.vector.tensor_scalar_mul(
                            v_blocks[:, g, :, 0:HD], vsrc,
                            rstdk8[:, g:g + 1])

                def first():
                    bcast()
                    group(cqT, qb_sb, qsh_sb, (qT01, qT23), "rbq", 0)()
                # order tuned for the phase1->attention handoff: hp=0 needs
                # qT01/kT01 first; v scales slot between so PV of wave 0 is
                # ready; hp=1 tensors land while hp=0's first waves run.
                chunks = [first,
                          group(ckvT, kb_sb, ksh_sb, (kT01, kT23), "rbkv", 0),
                          vtiles_mm, vtiles_scale,
                          group(cqT, qb_sb, qsh_sb, (qT01, qT23), "rbq", 1),
                          group(ckvT, kb_sb, ksh_sb, (kT01, kT23), "rbkv", 1)]
                return chunks

            # ================= Phase 1: down-projections + norms ==========
            # Per-block closure list so block sb+1's down-projection can be
            # interleaved as PE filler into block sb's attention waves
            # (phase 1 is HBM-feed-bound: ~11us per 2MB x block at the
            # 8-core-shared HBM rate).  cT tiles stay UNNORMALIZED; rstd_q
            # is applied post-rope in P2; rstd_kv/8 rides the exp scale
            # vector + V tile scale.  Ln->Exp per block costs ~2 act-table
            # swaps per block, mostly hidden in the attention exp stream.
            def ph1_block(sb):
                sl = slice(sb * SB, (sb + 1) * SB)
                st = {}

                def chunk(k):
                    def emit():
                        if k == 0:
                            st["cps"] = pair.tile([128, 2 * SB], F32,
                                                  name="cps", tag="ph1",
                                                  bufs=1)
                        cps = st["cps"]
                        xsl = xall[:,
                                   (sb * NKC + k) * SB:(sb * NKC + k + 1) * SB]
                        nc.tensor.matmul(cps[:, 0:SB],
                                         wkv_sb[:, k * KC:(k + 1) * KC], xsl,
                                         start=(k == 0), stop=(k == NKC - 1))
                        nc.tensor.matmul(cps[:, SB:2 * SB],
                                         wq_sb[:, k * KC:(k + 1) * KC], xsl,
                                         start=(k == 0), stop=(k == NKC - 1))
                    return emit

                def finish():
                    cps = st["cps"]
                    sq = sqp.tile([128, 2 * SB], BF16, name="sq", tag="sq")
                    st["sq"] = sq
                    nc.scalar.activation(sq[:], cps[:], AF.Square)
                    nc.vector.tensor_copy(ckvT[:, sl], cps[:, 0:SB])
                    nc.vector.tensor_copy(cqT[:, sl], cps[:, SB:2 * SB])

                def stats():
                    sq = st["sq"]
                    small = pair.tile([128, 2 * SB], F32, name="small",
                                      tag="ph1", bufs=1)
                    for t in range(4):
                        nc.tensor.matmul(small[:, SB + t:SB + t + 1],
                                         sq[:, t * ST:(t + 1) * ST],
                                         ones_col[:])
                    nc.tensor.matmul(small[0:1, 0:SB], ones_col[:],
                                     sq[:, SB:2 * SB])
                    if use_mask:
                        nc.tensor.matmul(small[32:33, 0:SB], ones_col[:],
                                         sq[:, 0:SB])
                    nc.scalar.activation(msT_all[:, sb * 4:(sb + 1) * 4],
                                         small[:, SB:SB + 4], AF.Ln,
                                         bias=eps_sb[:], scale=1.0 / R)
                    nc.scalar.activation(msq_row[0:1, sl], small[0:1, 0:SB],
                                         AF.Ln, bias=eps_sb[0:1, :],
                                         scale=1.0 / R)
                    if use_mask:
                        nc.scalar.activation(mskv_row[0:1, sl],
                                             small[32:33, 0:SB], AF.Ln,
                                             bias=eps_sb[0:1, :],
                                             scale=1.0 / R)
                    # per-block Exps: costs an Ln<->Exp act-table swap pair
                    # per block, but lets attention j=b start while later
                    # blocks' x is still streaming in (the bigger win).
                    nc.scalar.activation(rstdk8[:, sb * 4:(sb + 1) * 4],
                                         msT_all[:, sb * 4:(sb + 1) * 4],
                                         AF.Exp, scale=-0.5, bias=ln8_sb[:])
                    nc.scalar.activation(rstdq[0:1, sl], msq_row[0:1, sl],
                                         AF.Exp, scale=-0.5)
                    if use_mask:
                        nc.scalar.activation(rstdkv[0:1, sl],
                                             mskv_row[0:1, sl], AF.Exp,
                                             scale=-0.5)

                return [chunk(k) for k in range(NKC)] + [finish, stats]

            def merge_units(a, b):
                """Interleave 2 units of a per 1 of b, order-preserving."""
                out = []
                ia = ib = 0
                while ia < len(a) or ib < len(b):
                    for _ in range(2):
                        if ia < len(a):
                            out.append(a[ia])
                            ia += 1
                    if ib < len(b):
                        out.append(b[ib])
                        ib += 1
                return out

            def p4_units(j):
                units = []
                for t in range(4 * j, 4 * j + 4):
                    for dh in range(2):
                        def emit(t=t, dh=dh):
                            yp = pair.tile([128, 2 * SB], F32, name="yp",
                                           tag="pair")
                            # c outer: the outT01 (hp=0) contraction can
                            # issue before outT23's tail finishes
                            for c, oT in ((0, outT01), (1, outT23)):
                                for di in range(2):
                                    dcol = dh * 1024 + di * SB
                                    nc.tensor.matmul(
                                        yp[:, di * SB:(di + 1) * SB],
                                        oT[:, t * ST:(t + 1) * ST],
                                        wo_sb[:, c * D + dcol:c * D + dcol + SB],
                                        start=(c == 0), stop=(c == 1))
                            ysb = ysbp.tile([128, 2 * SB], F16, name="ysb",
                                            tag="ysb")
                            if dh == 0:
                                nc.scalar.activation(ysb[:], yp[:], AF.Copy)
                            else:
                                nc.vector.tensor_copy(ysb[:], yp[:])
                            nc.sync.dma_start(
                                y[t * ST:(t + 1) * ST,
                                  dh * 1024:(dh + 1) * 1024], ysb[:])
                        units.append(emit)
                return units

            def emit_scores(j, i, hp, mt):
                """Score matmuls + exp (+mask/tri) for key-tile i of block j,
                head pair hp. Returns the et tile."""
                q0 = ST * (i - 4 * j) if (causal and i >= 4 * j) else 0
                kTp, qTp = (kT01, qT01) if hp == 0 else (kT23, qT23)
                sc2 = pair.tile([128, 2 * SB], F32, name="sc2", tag="pair")
                for half in range(2):
                    hs = slice(half * 64, half * 64 + 64)
                    nc.tensor.matmul(
                        sc2[:, half * SB + q0:(half + 1) * SB],
                        kTp[hs, i * ST:(i + 1) * ST],
                        qTp[hs, j * SB + q0:(j + 1) * SB])
                if use_mask:
                    for half in range(2):
                        nc.vector.tensor_add(
                            sc2[:, half * SB:(half + 1) * SB],
                            sc2[:, half * SB:(half + 1) * SB], mt[:])
                et = etp.tile([128, 2 * SB], BF16, name="et", tag="et")
                scal = 0.125 if use_mask else rstdk8[:, i:i + 1]
                if q0 == 0:
                    nc.scalar.activation(et[:], sc2[:], AF.Exp, scale=scal)
                else:
                    for half in range(2):
                        rg = slice(half * SB + q0, (half + 1) * SB)
                        nc.scalar.activation(et[:, rg], sc2[:, rg],
                                             AF.Exp, scale=scal)
                if causal and i >= 4 * j:
                    for half in range(2):
                        rg = slice(half * SB + q0, half * SB + q0 + ST)
                        nc.vector.tensor_mul(et[:, rg], et[:, rg], tri_sb[:])
                return (et, q0)

            def emit_pv(j, i, hp, ets, first, last):
                et, q0 = ets
                for half in range(2):
                    h = 2 * hp + half
                    nc.tensor.matmul(
                        oacc2[half][:, q0:SB],
                        v_sb[:, i * (NH * VW) + h * VW:
                             i * (NH * VW) + (h + 1) * VW],
                        et[:, half * SB + q0:(half + 1) * SB],
                        start=first, stop=last)

            def emit_tail(j, hp):
                for half in range(2):
                    h = 2 * hp + half
                    den = rcp.tile([1, SB], F32, name="den", tag="den")
                    nc.vector.tensor_copy(den[:], oacc2[half][HD:VW, :])
                    rc = rcp.tile([1, SB], F32, name="rc", tag="rc")
                    nc.vector.reciprocal_approx_fast(out=rc[:], in_=den[:])
                    rbo = rbop.tile([HD, SB], F32, name="rbo", tag="rbo")
                    nc.gpsimd.partition_broadcast(rbo[:], rc[:])
                    dst = (outT01 if h < 2 else outT23)[
                        (h % 2) * HD:(h % 2 + 1) * HD,
                        j * SB:(j + 1) * SB]
                    nc.vector.tensor_mul(dst, oacc2[half][0:HD, :], rbo[:])

            # Phase 1 is HBM-feed-bound (~11us per 2MB x block at the
            # 8-core-shared HBM rate), so only block 0 runs up front;
            # blocks 1-3 are interleaved as PE filler into the attention
            # waves of the previous block (attention j=b needs only blocks
            # 0..b), hiding most of the x feed behind the softmax stream.
            blocks = [ph1_block(sb) for sb in range(NSB)]
            for c in blocks[0]:
                c()
            for c in p2_chunks(0):
                c()
            if not causal:
                for jj in range(1, NSB):
                    for c in blocks[jj]:
                        c()
                    for c in p2_chunks(jj):
                        c()

            def keep_warm(n):
                """Tiny matmuls on resident data bridging a known PE stall
                so the HAM clock-gate stays at K=8/8 (a >3.4us idle window
                re-throttles the PE to half clock, and re-warming takes
                another ~3.4us of sustained work)."""
                dummy = pair.tile([128, 2 * SB], F32, name="dummy",
                                  tag="pair")
                for _ in range(n):
                    nc.tensor.matmul(dummy[0:1, 0:ST], ones_col[:],
                                     ckvT[:, 0:ST])

            # bridge the upproj-rope DVE chain before block 0's first scores
            keep_warm(10)
            if not causal:
                for jj in range(1, NSB):
                    for c in p2_chunks(jj):
                        c()
            for j in range(NSB):
                ktiles = list(range(4 * j + 4)) if causal else list(range(NST))
                # fillers: next block's phase1+stats+up-proj chain, then
                # prev block's P4 units
                units = ((blocks[j + 1] + p2_chunks(j + 1))
                         if causal and j + 1 < NSB else []) \
                    + (p4_units(j - 1) if j > 0 else [])
                ui = 0
                nwaves = 2 * len(ktiles)
                wi = 0
                for hp in range(2):
                    oacc2 = [oaccp.tile([VW, SB], F32, name=f"oa{half}",
                                        tag="oa") for half in range(2)]
                    prev = None
                    for i in ktiles:
                        mt = None
                        if use_mask:
                            mt = mtp.tile([128, SB], F32, name="mt", tag="mt")
                            nc.sync.dma_start(
                                mt[:], maskT[i * ST:(i + 1) * ST,
                                             j * SB:(j + 1) * SB])
                        ets = emit_scores(j, i, hp, mt)
                        quota = -(-(len(units) - ui) // (nwaves - wi))
                        for _ in range(min(quota, 2)):
                            if ui < len(units):
                                units[ui]()
                                ui += 1
                        if prev is not None:
                            emit_pv(j, prev[0], hp, prev[1],
                                    prev[0] == ktiles[0], False)
                        prev = (i, ets)
                        wi += 1
                    emit_pv(j, prev[0], hp, prev[1],
                            prev[0] == ktiles[0], True)
                    emit_tail(j, hp)
                while ui < len(units):
                    units[ui]()
                    ui += 1
            # bridge the final tail chain so p4(NSB-1) runs at full clock
            keep_warm(24)
            for u in p4_units(NSB - 1):
                u()
            if dbg:
                for nm, t in (("d_ckvT", ckvT), ("d_cqT", cqT),
                              ("d_kT01", kT01), ("d_kT23", kT23),
                              ("d_qT01", qT01), ("d_qT23", qT23),
                              ("d_rstdq", rstdq), ("d_rstdk8", rstdk8),
                              ("d_vsb", v_sb)):
                    nc.sync.dma_start(dbg[nm], t[:])

    nc.finalize()
    return nc


_NC_CACHE = {}


def _get_nc(causal, use_mask):
    key = (causal, use_mask)
    if key not in _NC_CACHE:
        _NC_CACHE[key] = _build_nc(causal, use_mask)
    return _NC_CACHE[key]


def _prep_inputs(x, cos, sin, mask, w_kv_down, kv_norm_w, w_uk, w_ur, w_uv,
                 w_q_down, q_norm_w, w_uq, w_qr, w_o, use_mask):
    """Build the 8 per-core input maps (host-side shard + fold)."""
    import ml_dtypes as md
    f = np.float32
    x = np.asarray(x, f)
    cos = np.asarray(cos, f)
    sin = np.asarray(sin, f)
    w_kv_down = np.asarray(w_kv_down, f)
    w_q_down = np.asarray(w_q_down, f)
    kv_norm_w = np.asarray(kv_norm_w, f)
    q_norm_w = np.asarray(q_norm_w, f)
    w_uk_e = np.asarray(w_uk, f) * kv_norm_w[:, None]
    w_ur_e = np.asarray(w_ur, f) * kv_norm_w[:, None]
    w_uv_e = np.asarray(w_uv, f) * kv_norm_w[:, None]
    w_uq_e = np.asarray(w_uq, f) * q_norm_w[:, None]
    w_qr_e = np.asarray(w_qr, f) * q_norm_w[:, None]
    w_o = np.asarray(w_o, f)

    wkv = np.ascontiguousarray(
        w_kv_down.reshape(NKC, KC, R).transpose(1, 0, 2).reshape(KC, D))
    wq = np.ascontiguousarray(
        w_q_down.reshape(NKC, KC, R).transpose(1, 0, 2).reshape(KC, D))
    cosT = np.ascontiguousarray(cos.T)                 # [32, S]
    sinT = np.ascontiguousarray(sin.T)
    sinSg = np.concatenate([-sinT[:DR // 2], sinT[DR // 2:]], axis=0)
    one32 = np.ones((DR, S), np.float32)
    zero32 = np.zeros((DR, S), np.float32)
    # pair-tensor rope tables: nope rows pass through (cos=1, sin=0)
    cosPt = np.concatenate([one32, cosT, one32, cosT], axis=0)   # [128, S]
    sinPt = np.concatenate([zero32, sinSg, zero32, sinSg], axis=0)
    # s-block interleave: [cos_blk0 | sin_blk0 | cos_blk1 | sin_blk1 | ...]
    cssin = np.empty((128, 2 * S), np.float32)
    for sb in range(NSB):
        cssin[:, sb * 2 * SB:sb * 2 * SB + SB] = \
            cosPt[:, sb * SB:(sb + 1) * SB]
        cssin[:, sb * 2 * SB + SB:(sb + 1) * 2 * SB] = \
            sinPt[:, sb * SB:(sb + 1) * SB]
    cssin = np.ascontiguousarray(cssin).astype(md.bfloat16)
    # rope shift permutation within each head's 32 cols
    perm = np.concatenate([np.arange(16, 32), np.arange(0, 16)])

    # x: [b] -> transpose -> SBUF image [NSB, KC, NKC*SB]:
    # xT4[sb, p, k*SB + c] = xT[k*KC + p, sb*SB + c]
    xT4b = []
    for b in range(B):
        xT = x[b].T                                      # [D, S]
        xT4 = np.ascontiguousarray(
            xT.reshape(NKC, KC, NSB, SB).transpose(2, 1, 0, 3)
            .reshape(NSB, KC, NKC * SB)).astype(md.bfloat16)
        xT4b.append(xT4)
    maskT8 = None
    if use_mask:
        m = np.asarray(mask, f).reshape(S, S)
        maskT8 = np.ascontiguousarray(m.T) * 8.0

    in_maps = []
    z32 = np.zeros((R, DN), np.float32)
    for core in range(NCORES):
        b, g = core // 4, core % 4
        cs = slice(g * NH * DN, (g + 1) * NH * DN)      # 128-wide col slice
        vs = slice(g * NH * HD, (g + 1) * NH * HD)      # 256-wide
        uk_l = w_uk_e[:, cs].reshape(R, NH, DN)
        ur_l = w_ur_e[:, cs].reshape(R, NH, DR)
        urs_l = ur_l[:, :, perm]
        uq_l = w_uq_e[:, cs].reshape(R, NH, DN)
        qr_l = w_qr_e[:, cs].reshape(R, NH, DR)
        qrs_l = qr_l[:, :, perm]

        def pair(nope, rope):
            cols = []
            for h in range(NH):
                cols += [nope[:, h], rope[:, h]]
            return np.ascontiguousarray(np.concatenate(cols, axis=1))

        def pair_sh(sh):
            cols = []
            for h in range(NH):
                cols += [z32, sh[:, h]]
            return np.ascontiguousarray(np.concatenate(cols, axis=1))

        wo_loc = w_o[g * NH * HD:(g + 1) * NH * HD]     # [256, D]
        wo_r = np.ascontiguousarray(
            wo_loc.reshape(2, KC, D).transpose(1, 0, 2).reshape(KC, 2 * D)
        ).astype(md.bfloat16)
        m_ = {
            "xT4": xT4b[b],
            "wkv": wkv.astype(md.bfloat16), "wq": wq.astype(md.bfloat16),
            "kb": pair(uk_l, ur_l).astype(md.bfloat16),
            "ksh": pair_sh(urs_l).astype(md.bfloat16),
            "qb": pair(uq_l, qr_l).astype(md.bfloat16),
            "qsh": pair_sh(qrs_l).astype(md.bfloat16),
            "uv": np.ascontiguousarray(w_uv_e[:, vs]).astype(md.bfloat16),
            "wo": wo_r,
            "cssin": cssin,
        }
        if use_mask:
            m_["maskT"] = maskT8
        in_maps.append(m_)
    return in_maps


def _classify_mask(mask):
    m = np.asarray(mask, np.float32).reshape(S, S)
    if not np.any(m):
        return False, False          # dense, no mask
    causal_ref = np.where(
        np.tril(np.ones((S, S), dtype=bool)), np.float32(0.0),
        np.float32(-1e9))
    if np.array_equal(m, causal_ref):
        return True, False           # structural causal
    return False, True               # generic additive mask


LAST_RESULTS = None


def kernel(**inputs):
    global LAST_RESULTS
    from concourse.bass_utils import run_bass_kernel_spmd
    causal, use_mask = _classify_mask(inputs["mask"])
    nc = _get_nc(causal, use_mask)
    in_maps = _prep_inputs(
        inputs["x"], inputs["cos"], inputs["sin"], inputs["mask"],
        inputs["w_kv_down"], inputs["kv_norm_w"], inputs["w_uk"],
        inputs["w_ur"], inputs["w_uv"], inputs["w_q_down"],
        inputs["q_norm_w"], inputs["w_uq"], inputs["w_qr"], inputs["w_o"],
        use_mask)
    res = run_bass_kernel_spmd(nc, in_maps, list(range(NCORES)))
    LAST_RESULTS = res
    parts = [np.asarray(res.results[c]["y"], np.float32)
             for c in range(NCORES)]
    out = np.empty((B, S, D), np.float32)
    for b in range(B):
        out[b] = parts[4 * b] + parts[4 * b + 1] + parts[4 * b + 2] \
            + parts[4 * b + 3]
    return out



# revision 52
# speedup vs baseline: 1.1506x; 1.1506x over previous
"""DeepSeek-MLA forward kernel for 8 Trainium2 NeuronCores (Bass/Tile).

Sharding: core c -> batch b = c // 4, head-group g = c % 4 (4 of 16 heads).
Each core computes its batch's down-projections, its 4 heads' attention,
and a partial output projection; the host sums the 4 partials per batch.

Design notes (restructured from the 344us baseline; ~210us measured):
- x is staged in HBM as the exact SBUF image [NSB, 128, NKC*SB] and pulled
  in 512KB linear transfers (the 8 cores share HBM at ~180GB/s/core, and
  sub-1MB strided transfers waste half of that).
- Phase 1 (down-proj) runs as one uninterrupted PE stream: the norm-stat
  ones-matmuls for block b are deferred until after block b+1's matmuls so
  the PE never waits on the ACT Square.
- KV-side RMS norm is deferred: c_kv stays unnormalized; 1/rms(c_kv) enters
  as the per-partition (per-key) scale vector of the softmax exp and as the
  per-partition scale of the V tiles (folded with the 1/8 softmax
  temperature).  Q-side norm is applied post-rope with a gpsimd-broadcast
  row multiply.
- All Ln activations run inline during phase 1 and the Exps are lumped
  after (Ln and Exp live in different act-table sets; interleaving costs
  ~1.3us table reloads).  rstdk8's Exp goes first (unblocks V scales),
  then block 0's rstdq slice, so the transition chain is short.  The
  softmax reciprocal runs on DVE (reciprocal_approx_fast; PSUM and nonzero
  base partitions are unsupported by custom DVE ops).
- Up-projection chunk order (q0, k0, v-mm, v-scale, q1, k1) matches the
  DVE FIFO so rope multiplies never queue behind rstd-gated scales.
- Attention runs per HEAD-PAIR PASS: scores for 2 heads land in one
  [128,1024] two-bank PSUM tile (one exp ACTIVATE per pair); a 3-deep
  score-tile ring lets the PE run a full wave ahead of the ACT exp stream.
- The next block's up-projection/rope chunks and the previous block's
  output-projection units are interleaved into the attention waves as PE
  filler, eliminating block-boundary stalls (causal path; dense/mask paths
  hoist all phase-1/up-projection work before attention).
- y partials are stored fp16 (halves DMA); host sums in fp32.
"""

import os
import sys

import numpy as np

for _p in ("/opt/trn_rl_repo", "/root/.axon_site/_ro/trn_rl_repo"):
    if os.path.isdir(_p) and _p not in sys.path:
        sys.path.insert(0, _p)

import concourse.bass as bass
import concourse.mybir as mybir
import concourse.tile as tile
from concourse import bacc

B, S, D, H, DN, DR, R = 2, 2048, 2048, 16, 32, 32, 128
HD = DN + DR  # 64
EPS = 1e-5
NCORES = 8
NH = 4          # heads per core
SB = 512        # s-block (psum bank width in f32)
NSB = S // SB   # 4
ST = 128        # s-tile
NST = S // ST   # 16
KC = 128        # contraction chunk
NKC = D // KC   # 16
VW = HD + 1     # v columns incl. eighth column (65)
F32 = mybir.dt.float32
BF16 = mybir.dt.bfloat16
F16 = mybir.dt.float16
F8E4 = mybir.dt.float8e4
LN_EIGHTH = float(np.log(0.125))
WO_SCALE = 64.0      # wo is pre-scaled x64 on host (fp8 range)
OUT_SCALE = 8.0      # outT stored x8 (fp8 range); y copy divides by 512


def _build_nc(causal: bool, use_mask: bool):
    nc = bacc.Bacc("TRN2", target_bir_lowering=False, debug=False,
                   num_devices=NCORES)

    # x stored as the exact SBUF image: [NSB, 128, NKC*SB] so each s-block
    # is ONE fully-linear 2MB DMA (16KB per partition row) -- ~340GB/s vs
    # ~170GB/s for 128KB chunked transfers.
    xT4 = nc.dram_tensor("xT4", [NSB, KC, NKC * SB], BF16,
                         kind="ExternalInput").ap()
    wkv = nc.dram_tensor("wkv", [KC, D], BF16, kind="ExternalInput").ap()
    wq = nc.dram_tensor("wq", [KC, D], BF16, kind="ExternalInput").ap()
    kb = nc.dram_tensor("kb", [R, 2 * KC], BF16, kind="ExternalInput").ap()
    ksh = nc.dram_tensor("ksh", [R, 2 * KC], BF16, kind="ExternalInput").ap()
    qb = nc.dram_tensor("qb", [R, 2 * KC], BF16, kind="ExternalInput").ap()
    qsh = nc.dram_tensor("qsh", [R, 2 * KC], BF16, kind="ExternalInput").ap()
    uv = nc.dram_tensor("uv", [R, NH * HD], BF16, kind="ExternalInput").ap()
    wo = nc.dram_tensor("wo", [KC, 2 * D], BF16, kind="ExternalInput").ap()
    # rope tables, s-block interleaved: per sb, cols [0:512]=cos, [512:1024]=sin
    cssin = nc.dram_tensor("cssin", [128, 2 * S], BF16, kind="ExternalInput").ap()
    maskT = None
    if use_mask:
        maskT = nc.dram_tensor("maskT", [S, S], F32, kind="ExternalInput").ap()
    y = nc.dram_tensor("y", [S, D], F16, kind="ExternalOutput").ap()
    dbg = {}
    if os.environ.get("MLA_DEBUG"):
        for nm, shp, dt in (("d_ckvT", [R, S], BF16), ("d_cqT", [R, S], BF16),
                            ("d_kT01", [128, S], BF16), ("d_kT23", [128, S], BF16),
                            ("d_qT01", [128, S], BF16), ("d_qT23", [128, S], BF16),
                            ("d_rstdq", [1, S], F32), ("d_rstdk8", [128, NST], F32),
                            ("d_vsb", [128, NST * NH * VW], BF16),
                            ):
            dbg[nm] = nc.dram_tensor(nm, shp, dt, kind="ExternalOutput").ap()

    AF = mybir.ActivationFunctionType
    ALU = mybir.AluOpType

    with tile.TileContext(nc) as tc:
        from contextlib import ExitStack
        with ExitStack() as ctx:
            stat = ctx.enter_context(tc.tile_pool(name="static", bufs=1))
            # persistent SBUF tensors
            xall = stat.tile([128, NKC * S], BF16, name="xall")
            ckvT = stat.tile([R, S], BF16, name="ckvT")
            cqT = stat.tile([R, S], BF16, name="cqT")
            kT01 = stat.tile([128, S], BF16, name="kT01")
            kT23 = stat.tile([128, S], BF16, name="kT23")
            qT01 = stat.tile([128, S], BF16, name="qT01")
            qT23 = stat.tile([128, S], BF16, name="qT23")
            v_sb = stat.tile([128, NST * NH * VW], BF16, name="v_sb")
            outT01 = stat.tile([128, S], BF16, name="outT01")
            outT23 = stat.tile([128, S], BF16, name="outT23")
            wkv_sb = stat.tile([KC, D], BF16, name="wkv_sb")
            wq_sb = stat.tile([KC, D], BF16, name="wq_sb")
            kb_sb = stat.tile([R, 2 * KC], BF16, name="kb_sb")
            ksh_sb = stat.tile([R, 2 * KC], BF16, name="ksh_sb")
            qb_sb = stat.tile([R, 2 * KC], BF16, name="qb_sb")
            qsh_sb = stat.tile([R, 2 * KC], BF16, name="qsh_sb")
            uv_sb = stat.tile([R, NH * HD], BF16, name="uv_sb")
            wo_sb = stat.tile([KC, 2 * D], BF16, name="wo_sb")
            cssin_sb = stat.tile([128, 2 * S], BF16, name="cssin_sb")
            tri_sb = stat.tile([128, 128], BF16, name="tri_sb")
            ones_col = stat.tile([128, 1], BF16, name="ones_col")
            rstdk8 = stat.tile([128, NST], F32, name="rstdk8")
            rstdq = stat.tile([1, S], F32, name="rstdq")
            rstdkv = stat.tile([1, S], F32, name="rstdkv") if use_mask else None
            msq_row = stat.tile([1, S], F32, name="msq_row")
            mskv_row = stat.tile([1, S], F32, name="mskv_row") if use_mask \
                else None
            msT_all = stat.tile([128, NST], F32, name="msT_all")
            eps_sb = stat.tile([128, 1], F32, name="eps_sb")
            ln8_sb = stat.tile([128, 1], F32, name="ln8_sb")

            # ---- static loads, ordered by first use: wkv/wq + x block 0
            # feed phase 1 immediately; wo (first used ~40us in) goes last ----
            nc.sync.dma_start(wkv_sb[:], wkv)
            nc.sync.dma_start(wq_sb[:], wq)

            def load_x(sb, parts=1):
                w = NKC * SB // parts
                for p in range(parts):
                    nc.sync.dma_start(
                        xall[:, sb * NKC * SB + p * w:
                             sb * NKC * SB + (p + 1) * w],
                        xT4[sb, :, p * w:(p + 1) * w])
            # x in 4 sub-transfers per block so phase 1 streams
            for sb in range(NSB):
                load_x(sb, parts=4)
            nc.sync.dma_start(kb_sb[:], kb)
            nc.sync.dma_start(ksh_sb[:], ksh)
            nc.sync.dma_start(qb_sb[:], qb)
            nc.sync.dma_start(qsh_sb[:], qsh)
            nc.sync.dma_start(cssin_sb[:], cssin)
            nc.sync.dma_start(uv_sb[:], uv)
            nc.sync.dma_start(wo_sb[:], wo)

            nc.gpsimd.memset(ones_col[:], 1.0)
            nc.gpsimd.memset(tri_sb[:], 1.0)
            nc.gpsimd.affine_select(
                out=tri_sb[:], in_=tri_sb[:], compare_op=ALU.is_ge,
                fill=0.0, base=0, channel_multiplier=-1, pattern=[[1, 128]])
            nc.gpsimd.memset(eps_sb[:], EPS)
            nc.gpsimd.memset(ln8_sb[:], LN_EIGHTH)
            # v pre-filled with 0.125: PV accumulates denom/8; the DVE
            # reciprocal yields 8/denom, un-doing the 1/8 temperature that is
            # folded into the V scale.  (Last: it's big and not needed until
            # the first PV wave.)
            nc.gpsimd.memset(v_sb[:], 0.125)

            v_blocks = v_sb.rearrange("p (t h w) -> p t h w", t=NST, h=NH)

            # pools.  PSUM budget (8 banks): pair 3x2 + oacc 2x1.
            pair = ctx.enter_context(
                tc.tile_pool(name="pair", bufs=3, space="PSUM"))
            oaccp = ctx.enter_context(
                tc.tile_pool(name="oaccp", bufs=2, space="PSUM"))
            sqp = ctx.enter_context(tc.tile_pool(name="sqp", bufs=1 if use_mask else 2))
            t12p = ctx.enter_context(tc.tile_pool(name="t12p", bufs=1 if use_mask else 2))
            etp = ctx.enter_context(tc.tile_pool(name="etp", bufs=2 if use_mask else 4))
            rcp = ctx.enter_context(tc.tile_pool(name="rcp", bufs=2))
            tinyp = ctx.enter_context(tc.tile_pool(name="tinyp", bufs=2))
            rbqp = ctx.enter_context(tc.tile_pool(name="rbqp", bufs=2))
            rbop = ctx.enter_context(tc.tile_pool(name="rbop", bufs=2))
            ysbp = ctx.enter_context(tc.tile_pool(name="ysbp", bufs=1 if use_mask else 3))
            tailp = None if use_mask else ctx.enter_context(tc.tile_pool(name="tailp", bufs=1))
            ocp = ctx.enter_context(tc.tile_pool(name="ocp", bufs=2 if use_mask else 4))
            # mask path: reclaim 4kb from etp

            mtp = ctx.enter_context(tc.tile_pool(name="mtp", bufs=1)) \
                if use_mask else None

            # ================= Fused P2 -> P3 -> P4 pipeline ==============
            def p2_chunks(sb):
                """Up-projection + rope for block sb as filler closures, so
                block sb's kT/qT/v are produced during block sb-1's waves."""
                sl = slice(sb * SB, (sb + 1) * SB)
                cs_sl = cssin_sb[:, sb * 2 * SB:(sb + 1) * 2 * SB]
                state = {}

                def bcast():
                    rbq = rbqp.tile([128, SB], F32, name="rbq", tag="rbq")
                    nc.gpsimd.partition_broadcast(rbq[:], rstdq[0:1, sl])
                    state["rbq"] = rbq
                    if use_mask:
                        rbkv = rbqp.tile([128, SB], F32, name="rbkv",
                                         tag="rbq")
                        nc.gpsimd.partition_broadcast(rbkv[:],
                                                      rstdkv[0:1, sl])
                        state["rbkv"] = rbkv

                def group(cT, wb, wsh, dsts, rbkey, p):
                    def emit():
                        pp = pair.tile([128, 2 * SB], F32, name="pp",
                                       tag="pair")
                        nc.tensor.matmul(pp[:, 0:SB],
                                         wb[:, p * KC:(p + 1) * KC], cT[:, sl])
                        nc.tensor.matmul(pp[:, SB:2 * SB],
                                         wsh[:, p * KC:(p + 1) * KC], cT[:, sl])
                        t12 = t12p.tile([128, 2 * SB], F32, name="t12",
                                        tag="t12")
                        nc.vector.tensor_mul(t12[:], pp[:], cs_sl)
                        nc.vector.tensor_add(dsts[p][:, sl], t12[:, 0:SB],
                                             t12[:, SB:2 * SB])
                        rb = state.get(rbkey)
                        if rb is not None:
                            nc.vector.tensor_mul(dsts[p][:, sl],
                                                 dsts[p][:, sl], rb[:])
                    return emit

                def vtiles_mm():
                    vps = pair.tile([128, 2 * SB], F32, name="vps", tag="pair")
                    for t in range(4):
                        g = sb * 4 + t
                        nc.tensor.matmul(vps[:, t * 256:(t + 1) * 256],
                                         ckvT[:, g * ST:(g + 1) * ST],
                                         uv_sb[:])
                    state["vps"] = vps

                def vtiles_scale():
                    vps = state["vps"]
                    for t in range(4):
                        g = sb * 4 + t
                        vsrc = vps[:, t * 256:(t + 1) * 256].rearrange(
                            "p (h d) -> p h d", h=NH)
                        nc.vector.tensor_scalar_mul(
                            v_blocks[:, g, :, 0:HD], vsrc,
                            rstdk8[:, g:g + 1])

                def first():
                    bcast()
                    group(cqT, qb_sb, qsh_sb, (qT01, qT23), "rbq", 0)()
                # order tuned for the phase1->attention handoff: hp=0 needs
                # qT01/kT01 first; v scales slot between so PV of wave 0 is
                # ready; hp=1 tensors land while hp=0's first waves run.
                chunks = [first,
                          group(ckvT, kb_sb, ksh_sb, (kT01, kT23), "rbkv", 0),
                          vtiles_mm, vtiles_scale,
                          group(cqT, qb_sb, qsh_sb, (qT01, qT23), "rbq", 1),
                          group(ckvT, kb_sb, ksh_sb, (kT01, kT23), "rbkv", 1)]
                return chunks

            # ================= Phase 1: down-projections + norms ==========
            # Per-block closure list so block sb+1's down-projection can be
            # interleaved as PE filler into block sb's attention waves
            # (phase 1 is HBM-feed-bound: ~11us per 2MB x block at the
            # 8-core-shared HBM rate).  cT tiles stay UNNORMALIZED; rstd_q
            # is applied post-rope in P2; rstd_kv/8 rides the exp scale
            # vector + V tile scale.  Ln->Exp per block costs ~2 act-table
            # swaps per block, mostly hidden in the attention exp stream.
            def ph1_block(sb):
                sl = slice(sb * SB, (sb + 1) * SB)
                st = {}

                def chunk(k):
                    def emit():
                        if k == 0:
                            st["cps"] = pair.tile([128, 2 * SB], F32,
                                                  name="cps", tag="pair")
                        cps = st["cps"]
                        xsl = xall[:,
                                   (sb * NKC + k) * SB:(sb * NKC + k + 1) * SB]
                        nc.tensor.matmul(cps[:, 0:SB],
                                         wkv_sb[:, k * KC:(k + 1) * KC], xsl,
                                         start=(k == 0), stop=(k == NKC - 1))
                        nc.tensor.matmul(cps[:, SB:2 * SB],
                                         wq_sb[:, k * KC:(k + 1) * KC], xsl,
                                         start=(k == 0), stop=(k == NKC - 1))
                    return emit

                def finish():
                    cps = st["cps"]
                    sq = sqp.tile([128, 2 * SB], BF16, name="sq", tag="sq")
                    st["sq"] = sq
                    nc.scalar.activation(sq[:], cps[:], AF.Square)
                    nc.vector.tensor_copy(ckvT[:, sl], cps[:, 0:SB])
                    nc.vector.tensor_copy(cqT[:, sl], cps[:, SB:2 * SB])

                def stats():
                    sq = st["sq"]
                    small = pair.tile([128, 2 * SB], F32, name="small",
                                      tag="pair")
                    for t in range(4):
                        nc.tensor.matmul(small[:, SB + t:SB + t + 1],
                                         sq[:, t * ST:(t + 1) * ST],
                                         ones_col[:])
                    nc.tensor.matmul(small[0:1, 0:SB], ones_col[:],
                                     sq[:, SB:2 * SB])
                    if use_mask:
                        nc.tensor.matmul(small[32:33, 0:SB], ones_col[:],
                                         sq[:, 0:SB])
                    nc.scalar.activation(msT_all[:, sb * 4:(sb + 1) * 4],
                                         small[:, SB:SB + 4], AF.Ln,
                                         bias=eps_sb[:], scale=1.0 / R)
                    nc.scalar.activation(msq_row[0:1, sl], small[0:1, 0:SB],
                                         AF.Ln, bias=eps_sb[0:1, :],
                                         scale=1.0 / R)
                    if use_mask:
                        nc.scalar.activation(mskv_row[0:1, sl],
                                             small[32:33, 0:SB], AF.Ln,
                                             bias=eps_sb[0:1, :],
                                             scale=1.0 / R)

                return [chunk(k) for k in range(NKC)] + [finish, stats]

            def lumped_exps():
                # After every Ln (Ln and Exp live in different act-table
                # sets; interleaving costs ~1.3us reloads).  rstdk8 first
                # (unblocks V scales), then block 0's rstdq.
                nc.scalar.activation(rstdk8[:], msT_all[:], AF.Exp,
                                     scale=-0.5, bias=ln8_sb[:])
                nc.scalar.activation(rstdq[0:1, 0:SB], msq_row[0:1, 0:SB],
                                     AF.Exp, scale=-0.5)
                nc.scalar.activation(rstdq[0:1, SB:S], msq_row[0:1, SB:S],
                                     AF.Exp, scale=-0.5)
                if use_mask:
                    nc.scalar.activation(rstdkv[:], mskv_row[:], AF.Exp,
                                         scale=-0.5)

            def merge_units(a, b):
                """Interleave 2 units of a per 1 of b, order-preserving."""
                out = []
                ia = ib = 0
                while ia < len(a) or ib < len(b):
                    for _ in range(2):
                        if ia < len(a):
                            out.append(a[ia])
                            ia += 1
                    if ib < len(b):
                        out.append(b[ib])
                        ib += 1
                return out

            def p4_units(j):
                units = []
                for t in range(4 * j, 4 * j + 4):
                    for dh in range(2):
                        def emit(t=t, dh=dh):
                            yp = pair.tile([128, 2 * SB], F32, name="yp",
                                           tag="pair")
                            # c outer: the outT01 (hp=0) contraction can
                            # issue before outT23's tail finishes
                            for c, oT in ((0, outT01), (1, outT23)):
                                for di in range(2):
                                    dcol = dh * 1024 + di * SB
                                    nc.tensor.matmul(
                                        yp[:, di * SB:(di + 1) * SB],
                                        oT[:, t * ST:(t + 1) * ST],
                                        wo_sb[:, c * D + dcol:c * D + dcol + SB],
                                        start=(c == 0), stop=(c == 1))
                            ysb = ysbp.tile([128, 2 * SB], F16, name="ysb",
                                            tag="ysb")
                            if dh == 0:
                                nc.scalar.activation(ysb[:], yp[:], AF.Copy)
                            else:
                                nc.vector.tensor_copy(ysb[:], yp[:])
                            nc.sync.dma_start(
                                y[t * ST:(t + 1) * ST,
                                  dh * 1024:(dh + 1) * 1024], ysb[:])
                        units.append(emit)
                return units

            def emit_scores(j, i, hp, mt):
                """Score matmuls + exp (+mask/tri) for key-tile i of block j,
                head pair hp. Returns the et tile."""
                q0 = ST * (i - 4 * j) if (causal and i >= 4 * j) else 0
                kTp, qTp = (kT01, qT01) if hp == 0 else (kT23, qT23)
                sc2 = pair.tile([128, 2 * SB], F32, name="sc2", tag="pair")
                for half in range(2):
                    hs = slice(half * 64, half * 64 + 64)
                    nc.tensor.matmul(
                        sc2[:, half * SB + q0:(half + 1) * SB],
                        kTp[hs, i * ST:(i + 1) * ST],
                        qTp[hs, j * SB + q0:(j + 1) * SB])
                if use_mask:
                    for half in range(2):
                        nc.vector.tensor_add(
                            sc2[:, half * SB:(half + 1) * SB],
                            sc2[:, half * SB:(half + 1) * SB], mt[:])
                et = etp.tile([128, 2 * SB], BF16, name="et", tag="et")
                scal = 0.125 if use_mask else rstdk8[:, i:i + 1]
                if q0 == 0:
                    nc.scalar.activation(et[:], sc2[:], AF.Exp, scale=scal)
                else:
                    for half in range(2):
                        rg = slice(half * SB + q0, (half + 1) * SB)
                        nc.scalar.activation(et[:, rg], sc2[:, rg],
                                             AF.Exp, scale=scal)
                if causal and i >= 4 * j:
                    for half in range(2):
                        rg = slice(half * SB + q0, half * SB + q0 + ST)
                        nc.vector.tensor_mul(et[:, rg], et[:, rg], tri_sb[:])
                return (et, q0)

            def emit_pv(j, i, hp, ets, first, last):
                et, q0 = ets
                for half in range(2):
                    h = 2 * hp + half
                    nc.tensor.matmul(
                        oacc2[half][:, q0:SB],
                        v_sb[:, i * (NH * VW) + h * VW:
                             i * (NH * VW) + (h + 1) * VW],
                        et[:, half * SB + q0:(half + 1) * SB],
                        start=first, stop=last)

            def emit_tail(j, hp):
                for half in range(2):
                    h = 2 * hp + half
                    den = rcp.tile([1, SB], F32, name="den", tag="den")
                    nc.vector.tensor_copy(den[:], oacc2[half][HD:VW, :])
                    rc = rcp.tile([1, SB], F32, name="rc", tag="rc")
                    nc.vector.reciprocal_approx_fast(out=rc[:], in_=den[:])
                    rbo = rbop.tile([HD, SB], F32, name="rbo", tag="rbo")
                    nc.gpsimd.partition_broadcast(rbo[:], rc[:])
                    dst = (outT01 if h < 2 else outT23)[
                        (h % 2) * HD:(h % 2 + 1) * HD,
                        j * SB:(j + 1) * SB]
                    nc.vector.tensor_mul(dst, oacc2[half][0:HD, :], rbo[:])

            # Serial phase 1 (HBM-feed-bound), stats deferred one block so
            # the PE stream never waits on the ACT Square.
            blocks = [ph1_block(sb) for sb in range(NSB)]
            for sb in range(NSB):
                for c in blocks[sb][:NKC + 1]:   # chunks + finish
                    c()
                if sb >= 1:
                    blocks[sb - 1][NKC + 1]()    # stats(sb-1)
            blocks[NSB - 1][NKC + 1]()
            lumped_exps()
            for c in p2_chunks(0):
                c()

            def keep_warm(n):
                """Tiny matmuls on resident data bridging a known PE stall
                so the HAM clock-gate stays at K=8/8 (a >3.4us idle window
                re-throttles the PE to half clock, and re-warming takes
                another ~3.4us of sustained work)."""
                dummy = pair.tile([128, 2 * SB], F32, name="dummy",
                                  tag="pair")
                for _ in range(n):
                    nc.tensor.matmul(dummy[0:1, 0:ST], ones_col[:],
                                     ckvT[:, 0:ST])

            # bridge the upproj-rope DVE chain before block 0's first scores
            keep_warm(16)
            if not causal:
                for jj in range(1, NSB):
                    for c in p2_chunks(jj):
                        c()
            for j in range(NSB):
                ktiles = list(range(4 * j + 4)) if causal else list(range(NST))
                # fillers: next block's up-proj/rope first, then prev P4
                units = (p2_chunks(j + 1) if causal and j + 1 < NSB else []) \
                    + (p4_units(j - 1) if j > 0 else [])
                ui = 0
                nwaves = 2 * len(ktiles)
                wi = 0
                for hp in range(2):
                    oacc2 = [oaccp.tile([VW, SB], F32, name=f"oa{half}",
                                        tag="oa") for half in range(2)]
                    prev = None
                    for i in ktiles:
                        mt = None
                        if use_mask:
                            mt = mtp.tile([128, SB], F32, name="mt", tag="mt")
                            nc.sync.dma_start(
                                mt[:], maskT[i * ST:(i + 1) * ST,
                                             j * SB:(j + 1) * SB])
                        ets = emit_scores(j, i, hp, mt)
                        quota = -(-(len(units) - ui) // (nwaves - wi))
                        for _ in range(min(quota, 2)):
                            if ui < len(units):
                                units[ui]()
                                ui += 1
                        if prev is not None:
                            emit_pv(j, prev[0], hp, prev[1],
                                    prev[0] == ktiles[0], False)
                        prev = (i, ets)
                        wi += 1
                    emit_pv(j, prev[0], hp, prev[1],
                            prev[0] == ktiles[0], True)
                    emit_tail(j, hp)
                while ui < len(units):
                    units[ui]()
                    ui += 1
            # bridge the final tail chain so p4(NSB-1) runs at full clock
            keep_warm(24)
            for u in p4_units(NSB - 1):
                u()
            if dbg:
                for nm, t in (("d_ckvT", ckvT), ("d_cqT", cqT),
                              ("d_kT01", kT01), ("d_kT23", kT23),
                              ("d_qT01", qT01), ("d_qT23", qT23),
                              ("d_rstdq", rstdq), ("d_rstdk8", rstdk8),
                              ("d_vsb", v_sb)):
                    nc.sync.dma_start(dbg[nm], t[:])

    nc.finalize()
    return nc


_NC_CACHE = {}


def _get_nc(causal, use_mask):
    key = (causal, use_mask)
    if key not in _NC_CACHE:
        _NC_CACHE[key] = _build_nc(causal, use_mask)
    return _NC_CACHE[key]


def _prep_inputs(x, cos, sin, mask, w_kv_down, kv_norm_w, w_uk, w_ur, w_uv,
                 w_q_down, q_norm_w, w_uq, w_qr, w_o, use_mask):
    """Build the 8 per-core input maps (host-side shard + fold)."""
    import ml_dtypes as md
    f = np.float32
    x = np.asarray(x, f)
    cos = np.asarray(cos, f)
    sin = np.asarray(sin, f)
    w_kv_down = np.asarray(w_kv_down, f)
    w_q_down = np.asarray(w_q_down, f)
    kv_norm_w = np.asarray(kv_norm_w, f)
    q_norm_w = np.asarray(q_norm_w, f)
    w_uk_e = np.asarray(w_uk, f) * kv_norm_w[:, None]
    w_ur_e = np.asarray(w_ur, f) * kv_norm_w[:, None]
    w_uv_e = np.asarray(w_uv, f) * kv_norm_w[:, None]
    w_uq_e = np.asarray(w_uq, f) * q_norm_w[:, None]
    w_qr_e = np.asarray(w_qr, f) * q_norm_w[:, None]
    w_o = np.asarray(w_o, f)

    wkv = np.ascontiguousarray(
        w_kv_down.reshape(NKC, KC, R).transpose(1, 0, 2).reshape(KC, D))
    wq = np.ascontiguousarray(
        w_q_down.reshape(NKC, KC, R).transpose(1, 0, 2).reshape(KC, D))
    cosT = np.ascontiguousarray(cos.T)                 # [32, S]
    sinT = np.ascontiguousarray(sin.T)
    sinSg = np.concatenate([-sinT[:DR // 2], sinT[DR // 2:]], axis=0)
    one32 = np.ones((DR, S), np.float32)
    zero32 = np.zeros((DR, S), np.float32)
    # pair-tensor rope tables: nope rows pass through (cos=1, sin=0)
    cosPt = np.concatenate([one32, cosT, one32, cosT], axis=0)   # [128, S]
    sinPt = np.concatenate([zero32, sinSg, zero32, sinSg], axis=0)
    # s-block interleave: [cos_blk0 | sin_blk0 | cos_blk1 | sin_blk1 | ...]
    cssin = np.empty((128, 2 * S), np.float32)
    for sb in range(NSB):
        cssin[:, sb * 2 * SB:sb * 2 * SB + SB] = \
            cosPt[:, sb * SB:(sb + 1) * SB]
        cssin[:, sb * 2 * SB + SB:(sb + 1) * 2 * SB] = \
            sinPt[:, sb * SB:(sb + 1) * SB]
    cssin = np.ascontiguousarray(cssin).astype(md.bfloat16)
    # rope shift permutation within each head's 32 cols
    perm = np.concatenate([np.arange(16, 32), np.arange(0, 16)])

    # x: [b] -> transpose -> SBUF image [NSB, KC, NKC*SB]:
    # xT4[sb, p, k*SB + c] = xT[k*KC + p, sb*SB + c]
    xT4b = []
    for b in range(B):
        xT = x[b].T                                      # [D, S]
        xT4 = np.ascontiguousarray(
            xT.reshape(NKC, KC, NSB, SB).transpose(2, 1, 0, 3)
            .reshape(NSB, KC, NKC * SB)).astype(md.bfloat16)
        xT4b.append(xT4)
    maskT8 = None
    if use_mask:
        m = np.asarray(mask, f).reshape(S, S)
        maskT8 = np.ascontiguousarray(m.T) * 8.0

    in_maps = []
    z32 = np.zeros((R, DN), np.float32)
    for core in range(NCORES):
        b, g = core // 4, core % 4
        cs = slice(g * NH * DN, (g + 1) * NH * DN)      # 128-wide col slice
        vs = slice(g * NH * HD, (g + 1) * NH * HD)      # 256-wide
        uk_l = w_uk_e[:, cs].reshape(R, NH, DN)
        ur_l = w_ur_e[:, cs].reshape(R, NH, DR)
        urs_l = ur_l[:, :, perm]
        uq_l = w_uq_e[:, cs].reshape(R, NH, DN)
        qr_l = w_qr_e[:, cs].reshape(R, NH, DR)
        qrs_l = qr_l[:, :, perm]

        def pair(nope, rope):
            cols = []
            for h in range(NH):
                cols += [nope[:, h], rope[:, h]]
            return np.ascontiguousarray(np.concatenate(cols, axis=1))

        def pair_sh(sh):
            cols = []
            for h in range(NH):
                cols += [z32, sh[:, h]]
            return np.ascontiguousarray(np.concatenate(cols, axis=1))

        wo_loc = w_o[g * NH * HD:(g + 1) * NH * HD]     # [256, D]
        wo_r = np.ascontiguousarray(
            wo_loc.reshape(2, KC, D).transpose(1, 0, 2).reshape(KC, 2 * D)
        ).astype(md.bfloat16)
        m_ = {
            "xT4": xT4b[b],
            "wkv": wkv.astype(md.bfloat16), "wq": wq.astype(md.bfloat16),
            "kb": pair(uk_l, ur_l).astype(md.bfloat16),
            "ksh": pair_sh(urs_l).astype(md.bfloat16),
            "qb": pair(uq_l, qr_l).astype(md.bfloat16),
            "qsh": pair_sh(qrs_l).astype(md.bfloat16),
            "uv": np.ascontiguousarray(w_uv_e[:, vs]).astype(md.bfloat16),
            "wo": wo_r,
            "cssin": cssin,
        }
        if use_mask:
            m_["maskT"] = maskT8
        in_maps.append(m_)
    return in_maps


def _classify_mask(mask):
    m = np.asarray(mask, np.float32).reshape(S, S)
    if not np.any(m):
        return False, False          # dense, no mask
    causal_ref = np.where(
        np.tril(np.ones((S, S), dtype=bool)), np.float32(0.0),
        np.float32(-1e9))
    if np.array_equal(m, causal_ref):
        return True, False           # structural causal
    return False, True               # generic additive mask


LAST_RESULTS = None


def kernel(**inputs):
    global LAST_RESULTS
    from concourse.bass_utils import run_bass_kernel_spmd
    causal, use_mask = _classify_mask(inputs["mask"])
    nc = _get_nc(causal, use_mask)
    in_maps = _prep_inputs(
        inputs["x"], inputs["cos"], inputs["sin"], inputs["mask"],
        inputs["w_kv_down"], inputs["kv_norm_w"], inputs["w_uk"],
        inputs["w_ur"], inputs["w_uv"], inputs["w_q_down"],
        inputs["q_norm_w"], inputs["w_uq"], inputs["w_qr"], inputs["w_o"],
        use_mask)
    res = run_bass_kernel_spmd(nc, in_maps, list(range(NCORES)))
    LAST_RESULTS = res
    parts = [np.asarray(res.results[c]["y"], np.float32)
             for c in range(NCORES)]
    out = np.empty((B, S, D), np.float32)
    for b in range(B):
        out[b] = parts[4 * b] + parts[4 * b + 1] + parts[4 * b + 2] \
            + parts[4 * b + 3]
    return out

